# revision 15
# baseline (speedup 1.0000x reference)
"""GNN mean-aggregation message passing on 8 TRN2 NeuronCores.

out = x + 0.5 * segment_mean(x[src], dst)   for x [N, 128], edge_index [2, E]

Strategy (vertex-cut / destination partitioning):
  - Nodes (rows of x / out) are sharded row-wise across the 8 cores; each core
    receives the full x (replicated in its HBM, bf16 for the gather payload)
    plus only the edges whose destination lands in its row shard. No
    inter-core communication is needed: inputs are replicated and output
    shards are disjoint.
  - Edges are bucketed by (dst tile of 128 rows, src chunk of 25000 rows) and
    padded to groups of 128. The src chunking keeps gather indices within the
    int16 range of the SWDGE dma_gather ucode.
  - Per (octet of 8 dst tiles, chunk): one dma_gather pulls the 128-row edge
    groups of source features into SBUF. Gathers are striped across the 4
    SWDGE queues so their DMA-ring drains overlap (Q7 descriptor generation is
    the dominant serial cost of the gather).
  - A 0/1 selection matrix (built on DVE by comparing an iota row against each
    edge's dst slot) is the stationary matmul operand so the TensorEngine
    performs the segment-sum into PSUM (bf16 x bf16 -> f32 accumulate). The
    residual term is folded into the same accumulation as an extra matmul with
    a diagonal matrix of 2*max(deg,1), so the epilogue is a single
    per-row scale by 0.5/max(deg,1) followed by the output DMA.
"""

import sys

sys.path.insert(0, "/opt/trn_rl_repo")

import ml_dtypes
import numpy as np

import concourse.bass as bass
import concourse.bacc as bacc
import concourse.tile as tile
from concourse import mybir

P = 128  # partitions / tile rows / edge-group size
N_CORES = 8
CHUNK_ROWS = 25000  # src chunk size; must fit int16 index range
QUAD = 4  # dst tiles processed together (PSUM banks per quad)
N_QUEUES = 4  # SWDGE queues to stripe gathers over


def preprocess(x, edge_index, n_cores, chunk_rows=CHUNK_ROWS):
    """Shard/sort/pad edges; build per-core device input maps.

    Returns (per_core_inputs, G, n_rows, N, D) where G is a [T, n_chunks]
    array of 128-edge group counts per (dst tile, src chunk), shared across
    cores so the SPMD program is identical.
    """
    x = np.ascontiguousarray(x, dtype=np.float32)
    N, D = x.shape
    assert N % n_cores == 0
    n_rows = N // n_cores
    T = (n_rows + P - 1) // P  # dst tiles per core
    NC = (N + chunk_rows - 1) // chunk_rows  # src chunks
    assert chunk_rows <= 32767

    src = np.asarray(edge_index[0], dtype=np.int64)
    dst = np.asarray(edge_index[1], dtype=np.int64)

    core = dst // n_rows
    local = dst - core * n_rows
    tl = local >> 7  # dst tile within core
    ch = src // chunk_rows  # src chunk
    bucket = (core * T + tl) * NC + ch  # (core, tile, chunk)

    order = np.argsort(bucket, kind="stable")
    bucket_s = bucket[order]
    srcl_s = (src[order] - ch[order] * chunk_rows).astype(np.int16)
    dstl_s = (local[order] & 127).astype(np.float32)

    counts = np.bincount(bucket, minlength=n_cores * T * NC).reshape(n_cores, T, NC)
    # groups per (tile, chunk): max over cores so every core runs the same program
    G = ((counts.max(axis=0) + P - 1) // P).astype(np.int64)  # [T, NC]
    # every tile needs at least one group so its PSUM tile gets written
    empty = G.sum(axis=1) == 0
    G[empty, 0] = 1

    # stream order: octet-major, then chunk, then tile within octet
    n_quads = (T + QUAD - 1) // QUAD
    run_start = np.zeros((T, NC), dtype=np.int64)  # in groups
    off = 0
    for q in range(n_quads):
        tiles = range(q * QUAD, min((q + 1) * QUAD, T))
        for c in range(NC):
            for t in tiles:
                run_start[t, c] = off
                off += G[t, c]
    TG = int(off)  # total groups per core

    # position of each (sorted) edge inside its core's padded stream
    bucket_begin = np.concatenate([[0], np.cumsum(counts.reshape(-1))])[:-1]
    rank = np.arange(len(order)) - bucket_begin[bucket_s]
    t_of = (bucket_s // NC) % T
    c_of = bucket_s % NC
    pos = run_start[t_of, c_of] * P + rank
    core_s = bucket_s // (T * NC)

    deg_full = np.bincount(dst, minlength=N).astype(np.float32)
    iota = np.tile(
        np.arange(D, dtype=np.float32).astype(ml_dtypes.bfloat16), (P, 1)
    )  # [P, D] bf16 (values 0..127 are exact)
    xb = x.astype(ml_dtypes.bfloat16)

    per_core = []
    for c in range(n_cores):
        m = core_s == c
        src_pad = np.zeros(TG * P, dtype=np.int16)
        dstl_pad = np.full(TG * P, -1.0, dtype=np.float32)
        src_pad[pos[m]] = srcl_s[m]
        dstl_pad[pos[m]] = dstl_s[m]

        # gather indices: wrapped in 16 partitions, replicated to 128
        wrapped = np.ascontiguousarray(src_pad.reshape(TG * 8, 16).T)  # [16, TG*8]
        idxT = np.ascontiguousarray(np.tile(wrapped, (8, 1)))  # [128, TG*8]
        # dst slots: [TG, P] -> [P, TG] so partition p, col g holds edge g*128+p
        dstT = np.ascontiguousarray(
            dstl_pad.reshape(TG, P).T.astype(ml_dtypes.bfloat16)
        )

        deg_c = np.zeros(T * P, dtype=np.float32)
        deg_c[:n_rows] = deg_full[c * n_rows : (c + 1) * n_rows]
        degT = np.ascontiguousarray(deg_c.reshape(T, P).T)

        per_core.append(
            {
                "xb": xb,
                "xr": np.ascontiguousarray(x[c * n_rows : (c + 1) * n_rows]),
                "idxT": idxT,
                "dstT": dstT,
                "degT": degT,
                "iota": iota,
            }
        )
    return per_core, G, n_rows, N, D


def build_core_kernel(G, n_rows, N, D, chunk_rows=CHUNK_ROWS):
    """Build the per-core Bass program (identical across cores)."""
    T, NC = G.shape
    TG = int(G.sum())
    n_quads = (T + QUAD - 1) // QUAD
    f32 = mybir.dt.float32
    bf16 = mybir.dt.bfloat16
    i16 = mybir.dt.int16

    nc = bacc.Bacc("TRN2", target_bir_lowering=False, num_swdge_queues=N_QUEUES,
                   dynamic_dma_scratch_size=65536)

    xb_ext = nc.dram_tensor("xb", [N, D], bf16, kind="ExternalInput")
    xr_ext = nc.dram_tensor("xr", [n_rows, D], f32, kind="ExternalInput")
    idx_ext = nc.dram_tensor("idxT", [P, TG * 8], i16, kind="ExternalInput")
    dst_ext = nc.dram_tensor("dstT", [P, TG], bf16, kind="ExternalInput")
    deg_ext = nc.dram_tensor("degT", [P, T], f32, kind="ExternalInput")
    iota_ext = nc.dram_tensor("iota", [P, D], bf16, kind="ExternalInput")
    out_ext = nc.dram_tensor("out", [n_rows, D], f32, kind="ExternalOutput")

    with tile.TileContext(nc) as tc:
        with (
            tc.tile_pool(name="singles", bufs=1) as singles,
            tc.tile_pool(name="gather", bufs=8) as gpool,
            tc.tile_pool(name="onehot", bufs=4) as ohpool,
            tc.tile_pool(name="acc", bufs=2 * QUAD, space="PSUM") as psum,
            tc.tile_pool(name="xres", bufs=4) as xpool,
            tc.tile_pool(name="outp", bufs=4) as opool,
        ):
            idx_sb = singles.tile([P, TG * 8], i16)
            dst_sb = singles.tile([P, TG], bf16)
            deg_sb = singles.tile([P, T], f32)
            inv_sb = singles.tile([P, T], f32)

            iota_sb = singles.tile([P, D], bf16)

            nc.sync.dma_start(out=idx_sb[:], in_=idx_ext[:])
            nc.sync.dma_start(out=dst_sb[:], in_=dst_ext[:])
            nc.sync.dma_start(out=deg_sb[:], in_=deg_ext[:])
            nc.sync.dma_start(out=iota_sb[:], in_=iota_ext[:])

            # inv = 0.5 / max(deg, 1)
            nc.vector.tensor_scalar(
                out=inv_sb[:], in0=deg_sb[:], scalar1=1.0, scalar2=None,
                op0=mybir.AluOpType.max,
            )
            nc.vector.reciprocal(inv_sb[:], inv_sb[:])
            nc.vector.tensor_scalar(
                out=inv_sb[:], in0=inv_sb[:], scalar1=0.5, scalar2=None,
                op0=mybir.AluOpType.mult,
            )

            goff = 0
            n_gathers = 0
            for q in range(n_quads):
                tiles = list(range(q * QUAD, min((q + 1) * QUAD, T)))
                pts = {
                    t: psum.tile([P, D], f32, tag="acc", name=f"pt_{t}")
                    for t in tiles
                }
                seen = {t: 0 for t in tiles}
                tot = {t: int(G[t].sum()) for t in tiles}
                for c in range(NC):
                    S = int(sum(G[t, c] for t in tiles))
                    if S == 0:
                        continue
                    # one gather + one selection-matrix build for the segment
                    gt = gpool.tile([P, S, D], bf16, tag="gather")
                    nc.gpsimd.dma_gather(
                        out_ap=gt[:],
                        in_ap=xb_ext[c * chunk_rows : min(N, (c + 1) * chunk_rows), :],
                        idxs_ap=idx_sb[:, goff * 8 : (goff + S) * 8],
                        num_idxs=S * P,
                        num_idxs_reg=S * P,
                        elem_size=D,
                        single_packet=False,
                        queue_num=n_gathers % N_QUEUES,
                    )
                    n_gathers += 1
                    oh = ohpool.tile([P, S, D], bf16, tag="onehot")
                    iota_ap = iota_sb[:]
                    iota_b = bass.AP(
                        tensor=iota_ap.tensor,
                        offset=iota_ap.offset,
                        ap=[iota_ap.ap[0], [0, S], iota_ap.ap[1]],
                    )
                    dst_ap = dst_sb[:, goff : goff + S]
                    dst_b = bass.AP(
                        tensor=dst_ap.tensor,
                        offset=dst_ap.offset,
                        ap=[dst_ap.ap[0], dst_ap.ap[1], [0, D]],
                    )
                    nc.vector.tensor_tensor(
                        out=oh[:], in0=iota_b, in1=dst_b, op=mybir.AluOpType.is_equal
                    )

                    s = 0
                    for t in tiles:
                        for gi in range(int(G[t, c])):
                            seen[t] += 1
                            nc.tensor.matmul(
                                out=pts[t][:],
                                lhsT=oh[:, s, :],
                                rhs=gt[:, s, :],
                                start=(seen[t] == 1),
                                stop=(seen[t] == tot[t]),
                            )
                            s += 1
                    goff += S

                # epilogue: out = x + inv * acc
                for t in tiles:
                    rows = min(P, n_rows - t * P)
                    xt = xpool.tile([P, D], f32, tag="xres")
                    nc.sync.dma_start(
                        out=xt[:rows, :], in_=xr_ext[t * P : t * P + rows, :]
                    )
                    ot = opool.tile([P, D], f32, tag="outp")
                    nc.vector.tensor_scalar(
                        out=ot[:rows, :], in0=pts[t][:rows, :],
                        scalar1=inv_sb[:rows, t : t + 1], scalar2=None,
                        op0=mybir.AluOpType.mult,
                    )
                    nc.vector.tensor_tensor(
                        out=ot[:rows, :], in0=ot[:rows, :], in1=xt[:rows, :],
                        op=mybir.AluOpType.add,
                    )
                    nc.sync.dma_start(
                        out=out_ext[t * P : t * P + rows, :], in_=ot[:rows, :]
                    )

    nc.compile()
    return nc


def kernel(x, edge_index):
    from concourse.bass_utils import run_bass_kernel_spmd

    per_core, G, n_rows, N, D = preprocess(x, edge_index, N_CORES)
    nc = build_core_kernel(G, n_rows, N, D)
    res = run_bass_kernel_spmd(nc, per_core, core_ids=list(range(N_CORES)))
    out = np.concatenate([r["out"] for r in res.results], axis=0)
    return out.astype(np.float32)


# revision 17
# speedup vs baseline: 1.1321x; 1.1321x over previous
"""GNN mean-aggregation message passing on 8 TRN2 NeuronCores.

out = x + 0.5 * segment_mean(x[src], dst)   for x [N, 128], edge_index [2, E]

Strategy (vertex-cut / destination partitioning):
  - Nodes (rows of x / out) are sharded row-wise across the 8 cores; each core
    receives the full x (replicated in its HBM, bf16 for the gather payload)
    plus only the edges whose destination lands in its row shard. No
    inter-core communication is needed: inputs are replicated and output
    shards are disjoint.
  - Edges are bucketed by (dst tile of 128 rows, src chunk of 25000 rows) and
    padded to groups of 128. The src chunking keeps gather indices within the
    int16 range of the SWDGE dma_gather ucode.
  - Per (octet of 8 dst tiles, chunk): one dma_gather pulls the 128-row edge
    groups of source features into SBUF. Gathers are striped across the 4
    SWDGE queues so their DMA-ring drains overlap (Q7 descriptor generation is
    the dominant serial cost of the gather).
  - A 0/1 selection matrix (built on DVE by comparing an iota row against each
    edge's dst slot) is the stationary matmul operand so the TensorEngine
    performs the segment-sum into PSUM (bf16 x bf16 -> f32 accumulate). The
    residual term is folded into the same accumulation as an extra matmul with
    a diagonal matrix of 2*max(deg,1), so the epilogue is a single
    per-row scale by 0.5/max(deg,1) followed by the output DMA.
"""

import sys

sys.path.insert(0, "/opt/trn_rl_repo")

import ml_dtypes
import numpy as np

import concourse.bass as bass
import concourse.bacc as bacc
import concourse.tile as tile
from concourse import mybir

P = 128  # partitions / tile rows / edge-group size
N_CORES = 8
CHUNK_ROWS = 25000  # src chunk size; must fit int16 index range
QUAD = 2  # dst tiles processed together (PSUM banks per pair)
N_QUEUES = 4  # SWDGE queues to stripe gathers over


def preprocess(x, edge_index, n_cores, chunk_rows=CHUNK_ROWS):
    """Shard/sort/pad edges; build per-core device input maps.

    Returns (per_core_inputs, G, n_rows, N, D) where G is a [T, n_chunks]
    array of 128-edge group counts per (dst tile, src chunk), shared across
    cores so the SPMD program is identical.
    """
    x = np.ascontiguousarray(x, dtype=np.float32)
    N, D = x.shape
    assert N % n_cores == 0
    n_rows = N // n_cores
    T = (n_rows + P - 1) // P  # dst tiles per core
    NC = (N + chunk_rows - 1) // chunk_rows  # src chunks
    assert chunk_rows <= 32767

    src = np.asarray(edge_index[0], dtype=np.int64)
    dst = np.asarray(edge_index[1], dtype=np.int64)

    core = dst // n_rows
    local = dst - core * n_rows
    tl = local >> 7  # dst tile within core
    ch = src // chunk_rows  # src chunk
    bucket = (core * T + tl) * NC + ch  # (core, tile, chunk)

    order = np.argsort(bucket, kind="stable")
    bucket_s = bucket[order]
    srcl_s = (src[order] - ch[order] * chunk_rows).astype(np.int16)
    dstl_s = (local[order] & 127).astype(np.float32)

    counts = np.bincount(bucket, minlength=n_cores * T * NC).reshape(n_cores, T, NC)
    # groups per (tile, chunk): max over cores so every core runs the same program
    G = ((counts.max(axis=0) + P - 1) // P).astype(np.int64)  # [T, NC]
    # every tile needs at least one group so its PSUM tile gets written
    empty = G.sum(axis=1) == 0
    G[empty, 0] = 1

    # stream order: octet-major, then chunk, then tile within octet
    n_quads = (T + QUAD - 1) // QUAD
    run_start = np.zeros((T, NC), dtype=np.int64)  # in groups
    off = 0
    for q in range(n_quads):
        tiles = range(q * QUAD, min((q + 1) * QUAD, T))
        for c in range(NC):
            for t in tiles:
                run_start[t, c] = off
                off += G[t, c]
    TG = int(off)  # total groups per core

    # position of each (sorted) edge inside its core's padded stream
    bucket_begin = np.concatenate([[0], np.cumsum(counts.reshape(-1))])[:-1]
    rank = np.arange(len(order)) - bucket_begin[bucket_s]
    t_of = (bucket_s // NC) % T
    c_of = bucket_s % NC
    pos = run_start[t_of, c_of] * P + rank
    core_s = bucket_s // (T * NC)

    deg_full = np.bincount(dst, minlength=N).astype(np.float32)
    iota = np.tile(
        np.arange(D, dtype=np.float32).astype(ml_dtypes.bfloat16), (P, 1)
    )  # [P, D] bf16 (values 0..127 are exact)
    xb = x.astype(ml_dtypes.bfloat16)

    per_core = []
    for c in range(n_cores):
        m = core_s == c
        src_pad = np.zeros(TG * P, dtype=np.int16)
        dstl_pad = np.full(TG * P, -1.0, dtype=np.float32)
        src_pad[pos[m]] = srcl_s[m]
        dstl_pad[pos[m]] = dstl_s[m]

        # gather indices: wrapped in 16 partitions, replicated to 128
        wrapped = np.ascontiguousarray(src_pad.reshape(TG * 8, 16).T)  # [16, TG*8]
        idxT = np.ascontiguousarray(np.tile(wrapped, (8, 1)))  # [128, TG*8]
        # dst slots: [TG, P] -> [P, TG] so partition p, col g holds edge g*128+p
        dstT = np.ascontiguousarray(
            dstl_pad.reshape(TG, P).T.astype(ml_dtypes.bfloat16)
        )

        deg_c = np.zeros(T * P, dtype=np.float32)
        deg_c[:n_rows] = deg_full[c * n_rows : (c + 1) * n_rows]
        degT = np.ascontiguousarray(deg_c.reshape(T, P).T)

        per_core.append(
            {
                "xb": xb,
                "xr": np.ascontiguousarray(x[c * n_rows : (c + 1) * n_rows]),
                "idxT": idxT,
                "dstT": dstT,
                "degT": degT,
                "iota": iota,
            }
        )
    return per_core, G, n_rows, N, D


def build_core_kernel(G, n_rows, N, D, chunk_rows=CHUNK_ROWS):
    """Build the per-core Bass program (identical across cores)."""
    T, NC = G.shape
    TG = int(G.sum())
    n_quads = (T + QUAD - 1) // QUAD
    f32 = mybir.dt.float32
    bf16 = mybir.dt.bfloat16
    i16 = mybir.dt.int16

    nc = bacc.Bacc("TRN2", target_bir_lowering=False, num_swdge_queues=N_QUEUES)

    xb_ext = nc.dram_tensor("xb", [N, D], bf16, kind="ExternalInput")
    xr_ext = nc.dram_tensor("xr", [n_rows, D], f32, kind="ExternalInput")
    idx_ext = nc.dram_tensor("idxT", [P, TG * 8], i16, kind="ExternalInput")
    dst_ext = nc.dram_tensor("dstT", [P, TG], bf16, kind="ExternalInput")
    deg_ext = nc.dram_tensor("degT", [P, T], f32, kind="ExternalInput")
    iota_ext = nc.dram_tensor("iota", [P, D], bf16, kind="ExternalInput")
    out_ext = nc.dram_tensor("out", [n_rows, D], f32, kind="ExternalOutput")

    with tile.TileContext(nc) as tc:
        with (
            tc.tile_pool(name="singles", bufs=1) as singles,
            tc.tile_pool(name="gather", bufs=8) as gpool,
            tc.tile_pool(name="onehot", bufs=4) as ohpool,
            tc.tile_pool(name="acc", bufs=8, space="PSUM") as psum,
            tc.tile_pool(name="xres", bufs=4) as xpool,
            tc.tile_pool(name="outp", bufs=4) as opool,
        ):
            idx_sb = singles.tile([P, TG * 8], i16)
            dst_sb = singles.tile([P, TG], bf16)
            deg_sb = singles.tile([P, T], f32)
            inv_sb = singles.tile([P, T], f32)

            iota_sb = singles.tile([P, D], bf16)

            nc.sync.dma_start(out=idx_sb[:], in_=idx_ext[:])
            nc.sync.dma_start(out=dst_sb[:], in_=dst_ext[:])
            nc.sync.dma_start(out=deg_sb[:], in_=deg_ext[:])
            nc.sync.dma_start(out=iota_sb[:], in_=iota_ext[:])

            # inv = 0.5 / max(deg, 1)
            nc.vector.tensor_scalar(
                out=inv_sb[:], in0=deg_sb[:], scalar1=1.0, scalar2=None,
                op0=mybir.AluOpType.max,
            )
            nc.vector.reciprocal(inv_sb[:], inv_sb[:])
            nc.vector.tensor_scalar(
                out=inv_sb[:], in0=inv_sb[:], scalar1=0.5, scalar2=None,
                op0=mybir.AluOpType.mult,
            )

            goff = 0
            n_gathers = 0
            for q in range(n_quads):
                tiles = list(range(q * QUAD, min((q + 1) * QUAD, T)))
                pts = {
                    t: psum.tile([P, D], f32, tag="acc", name=f"pt_{t}")
                    for t in tiles
                }
                seen = {t: 0 for t in tiles}
                tot = {t: int(G[t].sum()) for t in tiles}
                for c in range(NC):
                    S = int(sum(G[t, c] for t in tiles))
                    if S == 0:
                        continue
                    # one gather + one selection-matrix build for the segment
                    gt = gpool.tile([P, S, D], bf16, tag="gather")
                    nc.gpsimd.dma_gather(
                        out_ap=gt[:],
                        in_ap=xb_ext[c * chunk_rows : min(N, (c + 1) * chunk_rows), :],
                        idxs_ap=idx_sb[:, goff * 8 : (goff + S) * 8],
                        num_idxs=S * P,
                        num_idxs_reg=S * P,
                        elem_size=D,
                        single_packet=False,
                        queue_num=n_gathers % N_QUEUES,
                    )
                    n_gathers += 1
                    oh = ohpool.tile([P, S, D], bf16, tag="onehot")
                    iota_ap = iota_sb[:]
                    iota_b = bass.AP(
                        tensor=iota_ap.tensor,
                        offset=iota_ap.offset,
                        ap=[iota_ap.ap[0], [0, S], iota_ap.ap[1]],
                    )
                    dst_ap = dst_sb[:, goff : goff + S]
                    dst_b = bass.AP(
                        tensor=dst_ap.tensor,
                        offset=dst_ap.offset,
                        ap=[dst_ap.ap[0], dst_ap.ap[1], [0, D]],
                    )
                    nc.vector.tensor_tensor(
                        out=oh[:], in0=iota_b, in1=dst_b, op=mybir.AluOpType.is_equal
                    )

                    s = 0
                    for t in tiles:
                        for gi in range(int(G[t, c])):
                            seen[t] += 1
                            nc.tensor.matmul(
                                out=pts[t][:],
                                lhsT=oh[:, s, :],
                                rhs=gt[:, s, :],
                                start=(seen[t] == 1),
                                stop=(seen[t] == tot[t]),
                            )
                            s += 1
                    goff += S

                # epilogue: out = x + inv * acc
                for t in tiles:
                    rows = min(P, n_rows - t * P)
                    xt = xpool.tile([P, D], f32, tag="xres")
                    nc.sync.dma_start(
                        out=xt[:rows, :], in_=xr_ext[t * P : t * P + rows, :]
                    )
                    ot = opool.tile([P, D], f32, tag="outp")
                    nc.vector.tensor_scalar(
                        out=ot[:rows, :], in0=pts[t][:rows, :],
                        scalar1=inv_sb[:rows, t : t + 1], scalar2=None,
                        op0=mybir.AluOpType.mult,
                    )
                    nc.vector.tensor_tensor(
                        out=ot[:rows, :], in0=ot[:rows, :], in1=xt[:rows, :],
                        op=mybir.AluOpType.add,
                    )
                    nc.sync.dma_start(
                        out=out_ext[t * P : t * P + rows, :], in_=ot[:rows, :]
                    )

    nc.compile()
    return nc


def kernel(x, edge_index):
    from concourse.bass_utils import run_bass_kernel_spmd

    per_core, G, n_rows, N, D = preprocess(x, edge_index, N_CORES)
    nc = build_core_kernel(G, n_rows, N, D)
    res = run_bass_kernel_spmd(nc, per_core, core_ids=list(range(N_CORES)))
    out = np.concatenate([r["out"] for r in res.results], axis=0)
    return out.astype(np.float32)
